# revision 19
# baseline (speedup 1.0000x reference)
"""Trainium2 Bass kernel for a custom Jacobi-basis layer.

Math:
    t = tanh(x)                                  x: [B, I] f32
    J[b,i,k] = P_k^(1,1)(t[b,i])                 Jacobi polys, k = 0..8
    out[b,o] = sum_{i,k} J[b,i,k] * coeff[o,i,k] * weights[o,i]

Strategy (8 NeuronCores, data-parallel over batch; ~42us vs the 45.3us
fp16 predecessor and 55.6us exact-recurrence baseline):
  * The matmul only needs SOME degree-graded polynomial basis of t; the
    Jacobi->device-basis change is folded into the host-prepared operand
    rho = Cw @ T.  Device basis = nested products of shifted squares:
        V1 = t        V2 = s = t*t    V3 = t*A      V4 = D*E
        V5 = V3*B     V6 = V4*F       V7 = V5*C     V8 = V6*G
    with X = alpha_X*(s - root_X).  Roots optimized (moment-matrix fit,
    tools/basis_opt8.py) to (a) near-orthogonalize the basis under the
    t=tanh(N(0,1)) measure (error amplification A^2 ~ 2.4 vs 31 for the
    old GAMMA knobs) and (b) concentrate the residual so planes V7+V8
    carry only ~9.5% of the output energy.
  * V7/V8 + their rho go fp8 (e4m3, scales balanced so neither operand
    falls into subnormals) and are contracted in a SINGLE DoubleRow
    matmul stream: both operands as [128, 2, n] APs (pair dim = plane),
    contraction 256 -> 2 planes per 216ns MM slot.  HW measured: pair
    MMs pace identically to fp16 N=512 MMs, and HW e4m3/DoubleRow
    numerics match the ml_dtypes simulation exactly.  Measured end to
    end rel err: 1.357e-2 vs the 2e-2 gate.  The other 6 planes + rho
    stay fp16.  PE stream: (6+2/2)*16 MMs @ 216ns ~= 24.2us (vs 27.6).
  * k=0 (J_0 == 1) is a bias added on the HOST after gathering; psum
    eviction is a plain copy, halves split across DVE and Scalar so the
    final tile's eviction+DMA chain is shorter.
  * Launch: first-needed operands land first.  scalar HW queue carries
    xt ic0/ic1 (64KB halves, triggers ahead of the tanh ACTIVATEs in NX
    order); sync queue carries r plane-1 chunks first, then xt ic2/ic3
    halves, r2/r3 halves, r4..r6, r78.  Early HWDGE queue throughput is
    slow and run-variable (~60-150GB/s ramping to 235GB/s; first chunk
    lands 2.3-3.5us after its trigger) -- this, plus the serial ~0.5us
    tanh ACTIVATEs, bounds the real stream start at ~10.4us.  N_WARM
    junk matmuls on a gpsimd-memset tile bridge the PE from ~7.1us so
    the HAM clock ramp completes (~9.9us) before real work.
  * exec_time_ns = (end of walrus teardown) - (first framework memset):
    the full semaphore-reset ladder (~7us: all 256 sems, Tensor engine
    slowest at ~135ns/reset) plus pre/post barriers are a fixed ~10us
    tail counted in the measurement.
  * x ships as fp16, out as fp16 (host casts up and adds bias).
"""

import numpy as np
import ml_dtypes

import concourse.mybir as mybir
import concourse.tile as tile
from concourse import bacc
from concourse.bass_utils import run_bass_kernel_spmd

ORDER = 8
B, I, O = 4096, 512, 512
NCORES = 8
BC = B // NCORES          # batch rows per core = 512
P = 128                   # partitions
NIC = I // P              # i-chunks = 4
BT = BC // P              # b-tiles per core = 4
FREE = NIC * BC           # free dim of basis planes = 2048
H = FREE // 2
NHEAVY = 6                # fp16 planes: t, s, V3..V6
HC = BC // 2              # xt half-chunk columns = 256

# Optimized nested-basis roots (see tools/basis_opt8.py): odd chain a,b,c;
# even chain d,e,f,g.  Light planes (V7, V8) carry 9.5% of output energy.
RA, RB, RC = 0.17514, 0.89828, 0.56799
RD, RE, RF, RG = 0.64867, 0.0357, 0.28938, 0.91742
# Per-factor scales: heavy planes max ~4 (fp16), fp8 planes max ~1.2/0.64
AL3, AL5, AL7 = 4.851, 40.718, 32.0
AL4, AL6, AL8 = 11.814, 16.629, 32.0
SC_A = AL3
SC_B = AL5 / AL3
SC_C = AL7 / AL5
SC_D = AL4
SC_E = 1.0
SC_F = AL6 / AL4
SC_G = AL8 / AL6

N_WARM = 14
LIGHT_FP8 = True


def _jacobi_t(t, order=ORDER, a=1.0, b=1.0):
    vals = [np.ones_like(t), 0.5 * (a + b + 2) * t - 0.5 * (a - b)]
    for i in range(2, order + 1):
        k1 = (2 * i + a + b) * (2 * i + a + b - 1) / (2 * i * (i + a + b))
        k3 = (i + a - 1) * (i + b - 1) * (2 * i + a + b) / (
            i * (i + a + b) * (2 * i + a + b - 2)
        )
        vals.append(k1 * t * vals[-1] - k3 * vals[-2])
    return np.stack(vals, axis=0)  # [order+1, n]


def _device_planes(t):
    """The 8 physical device planes (with scales), f64. t: any shape."""
    s = t * t
    A = SC_A * (s - RA)
    Bf = SC_B * (s - RB)
    Cf = SC_C * (s - RC)
    D = SC_D * (s - RD)
    Ef = SC_E * (s - RE)
    Ff = SC_F * (s - RF)
    Gf = SC_G * (s - RG)
    V3 = t * A
    V5 = V3 * Bf
    V7 = V5 * Cf
    V4 = D * Ef
    V6 = V4 * Ff
    V8 = V6 * Gf
    return [t, s, V3, V4, V5, V6, V7, V8]


def _basis_transform():
    """T[k,l] with J_k(t) = T[k,0]*1 + sum_l T[k,l] V_l(t)."""
    t = np.linspace(-0.999999, 0.999999, 8001)
    V = np.stack([np.ones_like(t)] + _device_planes(t), axis=0)  # [9, n]
    J = _jacobi_t(t)
    return np.linalg.lstsq(V.T, J.T, rcond=None)[0].T  # [9 k, 9 l]


def _build_module():
    nc = bacc.Bacc("TRN2", num_devices=NCORES)
    f32 = mybir.dt.float32
    f16 = mybir.dt.float16
    f8 = mybir.dt.float8e4
    mult = mybir.AluOpType.mult
    add = mybir.AluOpType.add
    subtract = mybir.AluOpType.subtract
    light_dt = f8 if LIGHT_FP8 else f16

    # xt ic-chunk-major: [ic, p, BC] fp16
    xt_d = nc.dram_tensor("xt", [NIC, P, BC], f16, kind="ExternalInput")
    # heavy rho plane-major: [l, p, ic*O + o], fp16
    r_d = nc.dram_tensor("r", [NHEAVY, P, FREE], f16, kind="ExternalInput")
    # light rho pair: [p, 2, ic*O + o] (dim1: 0=rho7, 1=rho8)
    r78_d = nc.dram_tensor("r78", [P, 2, FREE], light_dt, kind="ExternalInput")
    # out[bt, p, o] = output[core*BC + bt*128 + p, o] (fp16, host casts and
    # adds the k=0 bias term)
    out_d = nc.dram_tensor("out", [BT, P, O], f16, kind="ExternalOutput")

    with tile.TileContext(nc) as tc:
        with (
            tc.tile_pool(name="sb", bufs=1) as sb,
            tc.tile_pool(name="psum", bufs=1, space="PSUM") as pp,
        ):
            # --- PE warmup: memset on GpSimd (idle early; Vector would
            # delay nothing either, but GpSimd is free first).  Junk
            # matmuls bridge the HAM clock ramp until real operands land.
            warm_t = sb.tile([P, 256], f16, tag="warm")
            nc.gpsimd.memset(warm_t[:], 0.25)
            ps_warm = pp.tile([P, 256], f32, tag="warmps", name="ps_warm")
            for _ in range(N_WARM):
                nc.tensor.matmul(
                    ps_warm[:], warm_t[:, 0:P], warm_t[:, 0:256],
                    start=True, stop=True,
                )

            # --- tiles ---
            xt_t = sb.tile([P, FREE], f16, tag="xt")
            r_t = [
                sb.tile([P, FREE], f16, tag=f"r{l}", name=f"r{l}")
                for l in range(NHEAVY)
            ]
            r78_t = sb.tile([P, 2, FREE], light_dt, tag="r78")

            def xt_half(ic, h):
                lo = ic * BC + h * HC
                return xt_t[:, lo : lo + HC], xt_d[ic, :, h * HC : (h + 1) * HC]

            def xt_full(ic):
                return xt_t[:, ic * BC : (ic + 1) * BC], xt_d[ic]

            def r_chunk(l, ic):
                return (
                    r_t[l][:, ic * O : (ic + 1) * O],
                    r_d[l, :, ic * O : (ic + 1) * O],
                )

            def r_half(l, h):
                return r_t[l][:, h * H : (h + 1) * H], r_d[l, :, h * H : (h + 1) * H]

            # --- DMA enqueue ---
            # scalar HW data queue: xt ic0 + ic1 halves (tanh ACTIVATEs
            # interleave behind these triggers on the scalar NX)
            for ic in (0, 1):
                for h in (0, 1):
                    dst, src = xt_half(ic, h)
                    nc.scalar.dma_start(dst, src)

            # sync HW queue, in stream-consumption order; both queues share
            # the head load (each ramps slowly from ~1.5us first-byte).
            # r plane-1 ships as 8 64KB o-halves and plane-2 as 4 128KB
            # ic-chunks so the o-halved stream head consumes exactly at
            # arrival granularity.
            HO = O // 2
            for ic in range(NIC):
                for h in (0, 1):
                    lo = ic * O + h * HO
                    nc.sync.dma_start(
                        r_t[0][:, lo : lo + HO], r_d[0, :, lo : lo + HO]
                    )
            for ic in (2, 3):
                for h in (0, 1):
                    dst, src = xt_half(ic, h)
                    nc.sync.dma_start(dst, src)
            for ic in range(NIC):
                dst, src = r_chunk(1, ic)
                nc.sync.dma_start(dst, src)
            for h in (0, 1):
                dst, src = r_half(2, h)
                nc.sync.dma_start(dst, src)
            for l in range(3, NHEAVY):
                nc.sync.dma_start(r_t[l][:], r_d[l])
            nc.sync.dma_start(r78_t[:], r78_d[:])

            # --- tanh on scalar: fine halves for ic0/ic1 (early stream
            # deadlines), full chunks for ic2/ic3 (amortize the ~270ns
            # ACTIVATE fixed cost; their deadlines are later)
            t_t = sb.tile([P, FREE], f16, tag="t")
            Tanh = mybir.ActivationFunctionType.Tanh
            for ic in range(NIC):
                for hh in (0, 1):
                    lo = ic * BC + hh * HC
                    nc.scalar.activation(
                        t_t[:, lo : lo + HC], xt_t[:, lo : lo + HC], Tanh
                    )

            # --- basis planes on DVE ---
            s_t = sb.tile([P, FREE], f16, tag="s")
            fac = {
                k: sb.tile([P, FREE], f16, tag=f"fac{k}", name=f"fac{k}")
                for k in "ABCDEFG"
            }
            v3_t = sb.tile([P, FREE], f16, tag="v3")
            v4_t = sb.tile([P, FREE], f16, tag="v4")
            v5_t = sb.tile([P, FREE], f16, tag="v5")
            v6_t = sb.tile([P, FREE], f16, tag="v6")
            v78_t = sb.tile([P, 2, FREE], light_dt, tag="v78")
            halves = (slice(0, H), slice(H, FREE))

            # s = t*t in halves (earliest availability for plane 2)
            for h in (0, 1):
                nc.vector.tensor_tensor(
                    s_t[:, halves[h]], t_t[:, halves[h]], t_t[:, halves[h]], mult
                )

            def shift(dst, scale, root):
                # dst = scale*(s - root) = (s*scale) - (scale*root)
                nc.vector.tensor_scalar(
                    dst[:], s_t[:], float(scale), float(scale * root),
                    mult, subtract,
                )

            # A in halves so V3 halves can start as soon as s halves land
            for h in (0, 1):
                nc.vector.tensor_scalar(
                    fac["A"][:, halves[h]], s_t[:, halves[h]],
                    float(SC_A), float(SC_A * RA), mult, subtract,
                )
            for h in (0, 1):
                nc.vector.tensor_tensor(
                    v3_t[:, halves[h]], t_t[:, halves[h]], fac["A"][:, halves[h]],
                    mult,
                )
            shift(fac["D"], SC_D, RD)
            shift(fac["E"], SC_E, RE)
            nc.vector.tensor_tensor(v4_t[:], fac["D"][:], fac["E"][:], mult)
            shift(fac["B"], SC_B, RB)
            nc.vector.tensor_tensor(v5_t[:], v3_t[:], fac["B"][:], mult)
            shift(fac["F"], SC_F, RF)
            nc.vector.tensor_tensor(v6_t[:], v4_t[:], fac["F"][:], mult)
            shift(fac["C"], SC_C, RC)
            nc.vector.tensor_tensor(v78_t[:, 0, :], v5_t[:], fac["C"][:], mult)
            shift(fac["G"], SC_G, RG)
            nc.vector.tensor_tensor(v78_t[:, 1, :], v6_t[:], fac["G"][:], mult)

            planes = [t_t, s_t, v3_t, v4_t, v5_t, v6_t]

            # --- matmul stream ---
            psums = [
                pp.tile([P, O], f32, tag=f"ps{bt}", name=f"ps{bt}")
                for bt in range(BT)
            ]
            # plane 1 in o-halved MMs (N=256) matching the 64KB r1 chunks;
            # the bank-clear from each bank's first (h==0) MM leaves the h1
            # region's has_written bits unset, so h1's start=False first
            # write lands as overwrite-then-accumulate — correct.
            for ic in range(NIC):
                for h in (0, 1):
                    lo = ic * O + h * HO
                    for bt in range(BT):
                        col = ic * BC + bt * P
                        nc.tensor.matmul(
                            psums[bt][:, h * HO : (h + 1) * HO],
                            planes[0][:, col : col + P],
                            r_t[0][:, lo : lo + HO],
                            start=(ic == 0 and h == 0),
                            stop=False,
                        )
            for l in range(1, NHEAVY):
                for ic in range(NIC):
                    for bt in range(BT):
                        col = ic * BC + bt * P
                        nc.tensor.matmul(
                            psums[bt][:],
                            planes[l][:, col : col + P],
                            r_t[l][:, ic * O : (ic + 1) * O],
                            start=False,
                            stop=False,
                        )

            # light pair plane: DoubleRow fp8, bt-major so psum banks finish
            # one at a time; evict each bank's halves on DVE and Scalar in
            # parallel (bias is added on the host), fp16 out
            out_t = sb.tile([P, BT * O], f16, tag="out")
            Copy = mybir.ActivationFunctionType.Copy
            DR = mybir.MatmulPerfMode.DoubleRow if LIGHT_FP8 else None
            HO = O // 2
            for bt in range(BT):
                for ic in range(NIC):
                    col = ic * BC + bt * P
                    if LIGHT_FP8:
                        nc.tensor.matmul(
                            psums[bt][:],
                            v78_t[:, :, col : col + P],
                            r78_t[:, :, ic * O : (ic + 1) * O],
                            start=False,
                            stop=ic == NIC - 1,
                            perf_mode=DR,
                        )
                    else:
                        for pl in (0, 1):
                            nc.tensor.matmul(
                                psums[bt][:],
                                v78_t[:, pl, col : col + P],
                                r78_t[:, pl, ic * O : (ic + 1) * O],
                                start=False,
                                stop=(ic == NIC - 1 and pl == 1),
                            )
                nc.vector.tensor_copy(
                    out_t[:, bt * O : bt * O + HO], psums[bt][:, 0:HO]
                )
                nc.scalar.activation(
                    out_t[:, bt * O + HO : (bt + 1) * O], psums[bt][:, HO:O], Copy
                )
                if bt < BT - 1:
                    dma_eng = nc.sync if bt % 2 == 0 else nc.scalar
                    dma_eng.dma_start(out_d[bt], out_t[:, bt * O : (bt + 1) * O])
                else:
                    nc.sync.dma_start(
                        out_d[bt, :, 0:HO], out_t[:, bt * O : bt * O + HO]
                    )
                    nc.scalar.dma_start(
                        out_d[bt, :, HO:O], out_t[:, bt * O + HO : (bt + 1) * O]
                    )
    nc.compile()
    return nc


def _prep_operands(weights, coeff):
    """Host-side, input-independent preprocessing of the layer constants."""
    T = _basis_transform()
    Cw = coeff.astype(np.float64) * weights.astype(np.float64)[:, :, None]
    rho = np.einsum("oik,kl->oil", Cw, T)  # [O, I, 9]
    bias = rho[:, :, 0].sum(axis=1).astype(np.float32)  # [O], host-added

    def plane_layout(v, dt):
        # [O, I] -> [P, FREE] with [p, ic*O+o] = v[o, ic*128+p]
        tmp = v.T.astype(np.float32)  # [I, O]
        return tmp.reshape(NIC, P, O).transpose(1, 0, 2).reshape(P, FREE).astype(dt)

    r = np.empty((NHEAVY, P, FREE), dtype=np.float16)
    for l in range(NHEAVY):
        r[l] = plane_layout(rho[:, :, l + 1], np.float16)
    light_dt = ml_dtypes.float8_e4m3 if LIGHT_FP8 else np.float16
    r78 = np.empty((P, 2, FREE), dtype=light_dt)
    r78[:, 0, :] = plane_layout(np.clip(rho[:, :, 7], -224, 224), light_dt)
    r78[:, 1, :] = plane_layout(np.clip(rho[:, :, 8], -224, 224), light_dt)
    return (
        np.ascontiguousarray(r),
        np.ascontiguousarray(r78),
        bias,
    )


def _prep_x(x):
    """Per-core [NIC, 128, BC] fp16 views of x^T: xt[ic, p, b] = x[.., ic*128+p]."""
    shards = []
    for core in range(NCORES):
        xc = np.ascontiguousarray(x[core * BC : (core + 1) * BC, :].T)  # [I, BC]
        shards.append(
            np.ascontiguousarray(xc.reshape(NIC, P, BC)).astype(np.float16)
        )
    return shards


def _install_ntff_hook():
    """Register the NTFF profile hook that the image's boot skips (no
    antenv.axon_hooks module). Same ctypes ABI as trn_boot's
    _ntff_profile_via_ctypes. Only used for traced (profiling) runs."""
    import sys
    import types
    import ctypes
    import contextlib

    if "antenv.axon_hooks" in sys.modules:
        return
    mod = types.ModuleType("antenv.axon_hooks")
    state = {"hook": None}
    mod.set_axon_ntff_profile_hook = lambda h: state.__setitem__("hook", h)
    mod.get_axon_ntff_profile_hook = lambda: state["hook"]
    sys.modules["antenv.axon_hooks"] = mod
    import antenv

    antenv.axon_hooks = mod

    so_path = "/opt/axon/libaxon_pjrt.so"
    lib = ctypes.CDLL(so_path)
    if not hasattr(lib, "axon_start_nrt_profile"):
        return
    lib.axon_start_nrt_profile.argtypes = [
        ctypes.POINTER(ctypes.c_int64),
        ctypes.c_size_t,
    ]
    lib.axon_start_nrt_profile.restype = ctypes.c_int64
    lib.axon_stop_nrt_profile.argtypes = [ctypes.c_char_p]
    lib.axon_stop_nrt_profile.restype = ctypes.c_int64

    @contextlib.contextmanager
    def _hook(output_dir, device_ids):
        import jax

        jax.devices()
        if device_ids:
            ids = (ctypes.c_int64 * len(device_ids))(*device_ids)
            rc = lib.axon_start_nrt_profile(ids, len(device_ids))
        else:
            rc = lib.axon_start_nrt_profile(None, 0)
        if rc != 0:
            raise RuntimeError(f"axon_start_nrt_profile rc={rc}")
        try:
            yield
        finally:
            n = lib.axon_stop_nrt_profile(str(output_dir).encode())
            print(f"ntff profile: {n} file(s) written to {output_dir}")

    mod.set_axon_ntff_profile_hook(_hook)


_NC_CACHE = None


def _get_module():
    global _NC_CACHE
    if _NC_CACHE is None:
        _NC_CACHE = _build_module()
    return _NC_CACHE


def _run(x, weights, coeff, trace=False):
    nc = _get_module()
    r, r78, bias = _prep_operands(weights, coeff)
    xs = _prep_x(np.asarray(x, dtype=np.float32))
    in_maps = [
        {"xt": xs[core], "r": r, "r78": r78} for core in range(NCORES)
    ]
    try:
        res = run_bass_kernel_spmd(
            nc, in_maps, core_ids=list(range(NCORES)), trace=trace
        )
    except Exception:
        res = run_bass_kernel_spmd(
            nc, in_maps, core_ids=list(range(NCORES)), trace=trace
        )
    out = np.concatenate(
        [
            res.results[core]["out"].astype(np.float32).reshape(BC, O)
            for core in range(NCORES)
        ],
        axis=0,
    )
    out += bias[None, :]
    return out, res


def kernel(x, weights, coeff):
    out, _ = _run(
        np.asarray(x), np.asarray(weights), np.asarray(coeff), trace=False
    )
    return out


def kernel_traced(x, weights, coeff):
    _install_ntff_hook()
    out, res = _run(x, weights, coeff, trace=True)
    return out, res


# revision 22
# speedup vs baseline: 1.0683x; 1.0683x over previous
"""Trainium2 Bass kernel for a custom Jacobi-basis layer.

Math:
    t = tanh(x)                                  x: [B, I] f32
    J[b,i,k] = P_k^(1,1)(t[b,i])                 Jacobi polys, k = 0..8
    out[b,o] = sum_{i,k} J[b,i,k] * coeff[o,i,k] * weights[o,i]

Strategy (8 NeuronCores, data-parallel over batch; ~42us vs the 45.3us
fp16 predecessor and 55.6us exact-recurrence baseline):
  * The matmul only needs SOME degree-graded polynomial basis of t; the
    Jacobi->device-basis change is folded into the host-prepared operand
    rho = Cw @ T.  Device basis = nested products of shifted squares:
        V1 = t        V2 = s = t*t    V3 = t*A      V4 = D*E
        V5 = V3*B     V6 = V4*F       V7 = V5*C     V8 = V6*G
    with X = alpha_X*(s - root_X).  Roots optimized (moment-matrix fit,
    tools/basis_opt8.py) to (a) near-orthogonalize the basis under the
    t=tanh(N(0,1)) measure (error amplification A^2 ~ 2.4 vs 31 for the
    old GAMMA knobs) and (b) concentrate the residual so planes V7+V8
    carry only ~9.5% of the output energy.
  * V7/V8 + their rho go fp8 (e4m3, scales balanced so neither operand
    falls into subnormals) and are contracted in a SINGLE DoubleRow
    matmul stream: both operands as [128, 2, n] APs (pair dim = plane),
    contraction 256 -> 2 planes per 216ns MM slot.  HW measured: pair
    MMs pace identically to fp16 N=512 MMs, and HW e4m3/DoubleRow
    numerics match the ml_dtypes simulation exactly.  Measured end to
    end rel err: 1.357e-2 vs the 2e-2 gate.  The other 6 planes + rho
    stay fp16.  PE stream: (6+2/2)*16 MMs @ 216ns ~= 24.2us (vs 27.6).
  * k=0 (J_0 == 1) is a bias added on the HOST after gathering; psum
    eviction is a plain copy, halves split across DVE and Scalar so the
    final tile's eviction+DMA chain is shorter.
  * Launch: first-needed operands land first.  scalar HW queue carries
    xt ic0/ic1 (64KB halves, triggers ahead of the tanh ACTIVATEs in NX
    order); sync queue carries r plane-1 chunks first, then xt ic2/ic3
    halves, r2/r3 halves, r4..r6, r78.  Early HWDGE queue throughput is
    slow and run-variable (~60-150GB/s ramping to 235GB/s; first chunk
    lands 2.3-3.5us after its trigger) -- this, plus the serial ~0.5us
    tanh ACTIVATEs, bounds the real stream start at ~10.4us.  N_WARM
    junk matmuls on a gpsimd-memset tile bridge the PE from ~7.1us so
    the HAM clock ramp completes (~9.9us) before real work.
  * exec_time_ns = (end of walrus teardown) - (first framework memset):
    the full semaphore-reset ladder (~7us: all 256 sems, Tensor engine
    slowest at ~135ns/reset) plus pre/post barriers are a fixed ~10us
    tail counted in the measurement.
  * x ships as fp16, out as fp16 (host casts up and adds bias).
"""

import numpy as np
import ml_dtypes

import concourse.mybir as mybir
import concourse.tile as tile
from concourse import bacc
from concourse.bass_utils import run_bass_kernel_spmd

ORDER = 8
B, I, O = 4096, 512, 512
NCORES = 8
BC = B // NCORES          # batch rows per core = 512
P = 128                   # partitions
NIC = I // P              # i-chunks = 4
BT = BC // P              # b-tiles per core = 4
FREE = NIC * BC           # free dim of basis planes = 2048
H = FREE // 2
NHEAVY = 6                # fp16 planes: t, s, V3..V6
HC = BC // 2              # xt half-chunk columns = 256

# Optimized nested-basis roots (see tools/basis_opt8.py): odd chain a,b,c;
# even chain d,e,f,g.  Light planes (V7, V8) carry 9.5% of output energy.
RA, RB, RC = 0.17514, 0.89828, 0.56799
RD, RE, RF, RG = 0.64867, 0.0357, 0.28938, 0.91742
# Per-factor scales: heavy planes max ~4 (fp16), fp8 planes max ~1.2/0.64
AL3, AL5, AL7 = 4.851, 40.718, 32.0
AL4, AL6, AL8 = 11.814, 16.629, 32.0
SC_A = AL3
SC_B = AL5 / AL3
SC_C = AL7 / AL5
SC_D = AL4
SC_E = 1.0
SC_F = AL6 / AL4
SC_G = AL8 / AL6

N_WARM = 14
LIGHT_FP8 = True


def _jacobi_t(t, order=ORDER, a=1.0, b=1.0):
    vals = [np.ones_like(t), 0.5 * (a + b + 2) * t - 0.5 * (a - b)]
    for i in range(2, order + 1):
        k1 = (2 * i + a + b) * (2 * i + a + b - 1) / (2 * i * (i + a + b))
        k3 = (i + a - 1) * (i + b - 1) * (2 * i + a + b) / (
            i * (i + a + b) * (2 * i + a + b - 2)
        )
        vals.append(k1 * t * vals[-1] - k3 * vals[-2])
    return np.stack(vals, axis=0)  # [order+1, n]


def _device_planes(t):
    """The 8 physical device planes (with scales), f64. t: any shape."""
    s = t * t
    A = SC_A * (s - RA)
    Bf = SC_B * (s - RB)
    Cf = SC_C * (s - RC)
    D = SC_D * (s - RD)
    Ef = SC_E * (s - RE)
    Ff = SC_F * (s - RF)
    Gf = SC_G * (s - RG)
    V3 = t * A
    V5 = V3 * Bf
    V7 = V5 * Cf
    V4 = D * Ef
    V6 = V4 * Ff
    V8 = V6 * Gf
    return [t, s, V3, V4, V5, V6, V7, V8]


def _basis_transform():
    """T[k,l] with J_k(t) = T[k,0]*1 + sum_l T[k,l] V_l(t)."""
    t = np.linspace(-0.999999, 0.999999, 8001)
    V = np.stack([np.ones_like(t)] + _device_planes(t), axis=0)  # [9, n]
    J = _jacobi_t(t)
    return np.linalg.lstsq(V.T, J.T, rcond=None)[0].T  # [9 k, 9 l]


def _build_module():
    nc = bacc.Bacc("TRN2", num_devices=NCORES)
    f32 = mybir.dt.float32
    f16 = mybir.dt.float16
    f8 = mybir.dt.float8e4
    mult = mybir.AluOpType.mult
    add = mybir.AluOpType.add
    subtract = mybir.AluOpType.subtract
    light_dt = f8 if LIGHT_FP8 else f16

    # xt ic-chunk-major: [ic, p, BC] fp16
    xt_d = nc.dram_tensor("xt", [NIC, P, BC], f16, kind="ExternalInput")
    # heavy rho plane-major: [l, p, ic*O + o], fp16
    r_d = nc.dram_tensor("r", [NHEAVY, P, FREE], f16, kind="ExternalInput")
    # light rho pair: [p, 2, ic*O + o] (dim1: 0=rho7, 1=rho8)
    r78_d = nc.dram_tensor("r78", [P, 2, FREE], light_dt, kind="ExternalInput")
    # out[bt, p, o] = output[core*BC + bt*128 + p, o] (fp16, host casts and
    # adds the k=0 bias term)
    out_d = nc.dram_tensor("out", [BT, P, O], f16, kind="ExternalOutput")

    with tile.TileContext(nc) as tc:
        with (
            tc.tile_pool(name="sb", bufs=1) as sb,
            tc.tile_pool(name="psum", bufs=1, space="PSUM") as pp,
        ):
            # --- PE warmup: memset on GpSimd (idle early; Vector would
            # delay nothing either, but GpSimd is free first).  Junk
            # matmuls bridge the HAM clock ramp until real operands land.
            warm_t = sb.tile([P, 256], f16, tag="warm")
            nc.gpsimd.memset(warm_t[:], 0.25)
            ps_warm = pp.tile([P, 256], f32, tag="warmps", name="ps_warm")
            for _ in range(N_WARM):
                nc.tensor.matmul(
                    ps_warm[:], warm_t[:, 0:P], warm_t[:, 0:256],
                    start=True, stop=True,
                )

            # --- tiles ---
            xt_t = sb.tile([P, FREE], f16, tag="xt")
            r_t = [
                sb.tile([P, FREE], f16, tag=f"r{l}", name=f"r{l}")
                for l in range(NHEAVY)
            ]
            r78_t = sb.tile([P, 2, FREE], light_dt, tag="r78")

            def xt_half(ic, h):
                lo = ic * BC + h * HC
                return xt_t[:, lo : lo + HC], xt_d[ic, :, h * HC : (h + 1) * HC]

            def xt_full(ic):
                return xt_t[:, ic * BC : (ic + 1) * BC], xt_d[ic]

            def r_chunk(l, ic):
                return (
                    r_t[l][:, ic * O : (ic + 1) * O],
                    r_d[l, :, ic * O : (ic + 1) * O],
                )

            def r_half(l, h):
                return r_t[l][:, h * H : (h + 1) * H], r_d[l, :, h * H : (h + 1) * H]

            # --- DMA enqueue ---
            # scalar HW data queue: xt ic0 + ic1 halves (tanh ACTIVATEs
            # interleave behind these triggers on the scalar NX)
            for ic in (0, 1):
                for h in (0, 1):
                    dst, src = xt_half(ic, h)
                    nc.scalar.dma_start(dst, src)

            # sync HW queue, in stream-consumption order; both queues share
            # the head load (each ramps slowly from ~1.5us first-byte).
            # 128KB chunks for r1: smaller is descriptor-dominated (64KB
            # runs at ~138GB/s vs ~180+ at 128KB), bigger delays first use.
            for ic in range(NIC):
                dst, src = r_chunk(0, ic)
                nc.sync.dma_start(dst, src)
            for ic in (2, 3):
                for h in (0, 1):
                    dst, src = xt_half(ic, h)
                    nc.sync.dma_start(dst, src)
            for l in (1, 2):
                for h in (0, 1):
                    dst, src = r_half(l, h)
                    nc.sync.dma_start(dst, src)
            for l in range(3, NHEAVY):
                nc.sync.dma_start(r_t[l][:], r_d[l])
            nc.sync.dma_start(r78_t[:], r78_d[:])

            # --- tanh on scalar: fine halves for ic0/ic1 (early stream
            # deadlines), full chunks for ic2/ic3 (amortize the ~270ns
            # ACTIVATE fixed cost; their deadlines are later)
            t_t = sb.tile([P, FREE], f16, tag="t")
            Tanh = mybir.ActivationFunctionType.Tanh
            for ic in range(NIC):
                for hh in (0, 1):
                    lo = ic * BC + hh * HC
                    nc.scalar.activation(
                        t_t[:, lo : lo + HC], xt_t[:, lo : lo + HC], Tanh
                    )

            # --- basis planes on DVE ---
            s_t = sb.tile([P, FREE], f16, tag="s")
            fac = {
                k: sb.tile([P, FREE], f16, tag=f"fac{k}", name=f"fac{k}")
                for k in "ABCDEFG"
            }
            v3_t = sb.tile([P, FREE], f16, tag="v3")
            v4_t = sb.tile([P, FREE], f16, tag="v4")
            v5_t = sb.tile([P, FREE], f16, tag="v5")
            v6_t = sb.tile([P, FREE], f16, tag="v6")
            v78_t = sb.tile([P, 2, FREE], light_dt, tag="v78")
            halves = (slice(0, H), slice(H, FREE))

            # s = t*t in halves (earliest availability for plane 2)
            for h in (0, 1):
                nc.vector.tensor_tensor(
                    s_t[:, halves[h]], t_t[:, halves[h]], t_t[:, halves[h]], mult
                )

            def shift(dst, scale, root):
                # dst = scale*(s - root) = (s*scale) - (scale*root)
                nc.vector.tensor_scalar(
                    dst[:], s_t[:], float(scale), float(scale * root),
                    mult, subtract,
                )

            # A in halves so V3 halves can start as soon as s halves land
            for h in (0, 1):
                nc.vector.tensor_scalar(
                    fac["A"][:, halves[h]], s_t[:, halves[h]],
                    float(SC_A), float(SC_A * RA), mult, subtract,
                )
            for h in (0, 1):
                nc.vector.tensor_tensor(
                    v3_t[:, halves[h]], t_t[:, halves[h]], fac["A"][:, halves[h]],
                    mult,
                )
            shift(fac["D"], SC_D, RD)
            shift(fac["E"], SC_E, RE)
            nc.vector.tensor_tensor(v4_t[:], fac["D"][:], fac["E"][:], mult)
            shift(fac["B"], SC_B, RB)
            nc.vector.tensor_tensor(v5_t[:], v3_t[:], fac["B"][:], mult)
            shift(fac["F"], SC_F, RF)
            nc.vector.tensor_tensor(v6_t[:], v4_t[:], fac["F"][:], mult)
            shift(fac["C"], SC_C, RC)
            nc.vector.tensor_tensor(v78_t[:, 0, :], v5_t[:], fac["C"][:], mult)
            shift(fac["G"], SC_G, RG)
            nc.vector.tensor_tensor(v78_t[:, 1, :], v6_t[:], fac["G"][:], mult)

            planes = [t_t, s_t, v3_t, v4_t, v5_t, v6_t]

            # --- matmul stream ---
            psums = [
                pp.tile([P, O], f32, tag=f"ps{bt}", name=f"ps{bt}")
                for bt in range(BT)
            ]
            for l in range(NHEAVY):
                for ic in range(NIC):
                    for bt in range(BT):
                        col = ic * BC + bt * P
                        nc.tensor.matmul(
                            psums[bt][:],
                            planes[l][:, col : col + P],
                            r_t[l][:, ic * O : (ic + 1) * O],
                            start=(l == 0 and ic == 0),
                            stop=False,
                        )

            # light pair plane: DoubleRow fp8, bt-major so psum banks finish
            # one at a time; evict each bank's halves on DVE and Scalar in
            # parallel (bias is added on the host), fp16 out
            out_t = sb.tile([P, BT * O], f16, tag="out")
            Copy = mybir.ActivationFunctionType.Copy
            DR = mybir.MatmulPerfMode.DoubleRow if LIGHT_FP8 else None
            HO = O // 2
            for bt in range(BT):
                for ic in range(NIC):
                    col = ic * BC + bt * P
                    if LIGHT_FP8:
                        nc.tensor.matmul(
                            psums[bt][:],
                            v78_t[:, :, col : col + P],
                            r78_t[:, :, ic * O : (ic + 1) * O],
                            start=False,
                            stop=ic == NIC - 1,
                            perf_mode=DR,
                        )
                    else:
                        for pl in (0, 1):
                            nc.tensor.matmul(
                                psums[bt][:],
                                v78_t[:, pl, col : col + P],
                                r78_t[:, pl, ic * O : (ic + 1) * O],
                                start=False,
                                stop=(ic == NIC - 1 and pl == 1),
                            )
                nc.vector.tensor_copy(
                    out_t[:, bt * O : bt * O + HO], psums[bt][:, 0:HO]
                )
                nc.scalar.activation(
                    out_t[:, bt * O + HO : (bt + 1) * O], psums[bt][:, HO:O], Copy
                )
                if bt < BT - 1:
                    dma_eng = nc.sync if bt % 2 == 0 else nc.scalar
                    dma_eng.dma_start(out_d[bt], out_t[:, bt * O : (bt + 1) * O])
                else:
                    nc.sync.dma_start(
                        out_d[bt, :, 0:HO], out_t[:, bt * O : bt * O + HO]
                    )
                    nc.scalar.dma_start(
                        out_d[bt, :, HO:O], out_t[:, bt * O + HO : (bt + 1) * O]
                    )
    nc.compile()
    return nc


def _prep_operands(weights, coeff):
    """Host-side, input-independent preprocessing of the layer constants."""
    T = _basis_transform()
    Cw = coeff.astype(np.float64) * weights.astype(np.float64)[:, :, None]
    rho = np.einsum("oik,kl->oil", Cw, T)  # [O, I, 9]
    bias = rho[:, :, 0].sum(axis=1).astype(np.float32)  # [O], host-added

    def plane_layout(v, dt):
        # [O, I] -> [P, FREE] with [p, ic*O+o] = v[o, ic*128+p]
        tmp = v.T.astype(np.float32)  # [I, O]
        return tmp.reshape(NIC, P, O).transpose(1, 0, 2).reshape(P, FREE).astype(dt)

    r = np.empty((NHEAVY, P, FREE), dtype=np.float16)
    for l in range(NHEAVY):
        r[l] = plane_layout(rho[:, :, l + 1], np.float16)
    light_dt = ml_dtypes.float8_e4m3 if LIGHT_FP8 else np.float16
    r78 = np.empty((P, 2, FREE), dtype=light_dt)
    r78[:, 0, :] = plane_layout(np.clip(rho[:, :, 7], -224, 224), light_dt)
    r78[:, 1, :] = plane_layout(np.clip(rho[:, :, 8], -224, 224), light_dt)
    return (
        np.ascontiguousarray(r),
        np.ascontiguousarray(r78),
        bias,
    )


def _prep_x(x):
    """Per-core [NIC, 128, BC] fp16 views of x^T: xt[ic, p, b] = x[.., ic*128+p]."""
    shards = []
    for core in range(NCORES):
        xc = np.ascontiguousarray(x[core * BC : (core + 1) * BC, :].T)  # [I, BC]
        shards.append(
            np.ascontiguousarray(xc.reshape(NIC, P, BC)).astype(np.float16)
        )
    return shards


def _install_ntff_hook():
    """Register the NTFF profile hook that the image's boot skips (no
    antenv.axon_hooks module). Same ctypes ABI as trn_boot's
    _ntff_profile_via_ctypes. Only used for traced (profiling) runs."""
    import sys
    import types
    import ctypes
    import contextlib

    if "antenv.axon_hooks" in sys.modules:
        return
    mod = types.ModuleType("antenv.axon_hooks")
    state = {"hook": None}
    mod.set_axon_ntff_profile_hook = lambda h: state.__setitem__("hook", h)
    mod.get_axon_ntff_profile_hook = lambda: state["hook"]
    sys.modules["antenv.axon_hooks"] = mod
    import antenv

    antenv.axon_hooks = mod

    so_path = "/opt/axon/libaxon_pjrt.so"
    lib = ctypes.CDLL(so_path)
    if not hasattr(lib, "axon_start_nrt_profile"):
        return
    lib.axon_start_nrt_profile.argtypes = [
        ctypes.POINTER(ctypes.c_int64),
        ctypes.c_size_t,
    ]
    lib.axon_start_nrt_profile.restype = ctypes.c_int64
    lib.axon_stop_nrt_profile.argtypes = [ctypes.c_char_p]
    lib.axon_stop_nrt_profile.restype = ctypes.c_int64

    @contextlib.contextmanager
    def _hook(output_dir, device_ids):
        import jax

        jax.devices()
        if device_ids:
            ids = (ctypes.c_int64 * len(device_ids))(*device_ids)
            rc = lib.axon_start_nrt_profile(ids, len(device_ids))
        else:
            rc = lib.axon_start_nrt_profile(None, 0)
        if rc != 0:
            raise RuntimeError(f"axon_start_nrt_profile rc={rc}")
        try:
            yield
        finally:
            n = lib.axon_stop_nrt_profile(str(output_dir).encode())
            print(f"ntff profile: {n} file(s) written to {output_dir}")

    mod.set_axon_ntff_profile_hook(_hook)


_NC_CACHE = None


def _get_module():
    global _NC_CACHE
    if _NC_CACHE is None:
        _NC_CACHE = _build_module()
    return _NC_CACHE


def _run(x, weights, coeff, trace=False):
    nc = _get_module()
    r, r78, bias = _prep_operands(weights, coeff)
    xs = _prep_x(np.asarray(x, dtype=np.float32))
    in_maps = [
        {"xt": xs[core], "r": r, "r78": r78} for core in range(NCORES)
    ]
    def run_once():
        return run_bass_kernel_spmd(
            nc, in_maps, core_ids=list(range(NCORES)), trace=trace
        )

    def gather(res):
        return np.concatenate(
            [
                res.results[core]["out"].astype(np.float32).reshape(BC, O)
                for core in range(NCORES)
            ],
            axis=0,
        )

    try:
        res = run_once()
    except Exception:
        res = run_once()
    out = gather(res)
    if not np.isfinite(out).all():
        # rare transient hardware flake: retry once
        res = run_once()
        out = gather(res)
    out += bias[None, :]
    return out, res


def kernel(x, weights, coeff):
    out, _ = _run(
        np.asarray(x), np.asarray(weights), np.asarray(coeff), trace=False
    )
    return out


def kernel_traced(x, weights, coeff):
    _install_ntff_hook()
    out, res = _run(x, weights, coeff, trace=True)
    return out, res
